# revision 7
# baseline (speedup 1.0000x reference)
"""Trainium2 Bass kernel for nn_Inter_RM_6940667150684 (gnn_message_passing).

Math (per example n):
  g[n,m,:] = relu(f[n,m,:] @ W[m].T)            (W[m,e,d], contract d)
  s[n,j,k] = ||g_j - g_k||^2
  edges    = tanh(sqrt(relu(s)))  (diag 0, symmetric)
  y[n]     = 0.5*sum_m f[n,m,:] + sum_k c_k[n]*g[n,k,:]
  c_k[n]   = 0.5*sum_{j!=k} tanh(sqrt(relu(s_jk)))

Sharding: pure data parallel over batch N=8192 -> 8 cores x 1024 rows.
"""

import sys

sys.path.insert(0, "/opt/trn_rl_repo")

import numpy as np

N, M, D, E = 8192, 9, 128, 128
NCORES = 8
NLOC = N // NCORES          # 1024 rows per core
NT = NLOC // 128            # 8 ntiles of 128 examples
NPAIR = 45                  # triangular tiles (j<=k), squares included
NP2 = 36                    # strict pairs j<k
NG = 12                     # dot-mm groups of 4 tiles

_TRI = [(j, k) for j in range(M) for k in range(j, M)]
_OFF = [0]
for j in range(M):
    _OFF.append(_OFF[-1] + (M - j))
_SQ = [_OFF[j] for j in range(M)]                       # index of (j,j)
_PAIRS = [(t, jk) for t, jk in enumerate(_TRI) if jk[0] < jk[1]]


def _host_consts():
    e4 = np.zeros((D, NG, NPAIR), np.float32)
    for t in range(NPAIR):
        e4[:, t // 4, t] = 1.0
    maddT = np.zeros((NPAIR, 4, NP2), np.float32)
    minc = np.zeros((NP2, M), np.float32)
    for p, (t, (j, k)) in enumerate(_PAIRS):
        maddT[_SQ[j], _SQ[j] % 4, p] += 1.0
        maddT[_SQ[k], _SQ[k] % 4, p] += 1.0
        maddT[t, t % 4, p] += -2.0
        minc[p, j] = 0.5
        minc[p, k] = 0.5
    eye = np.eye(128, dtype=np.float32)
    return e4, maddT, minc, eye


def _emit(nc):
    from concourse import bass, tile
    from concourse.alu_op_type import AluOpType

    mybir = bass.mybir
    FP32 = mybir.dt.float32
    BF16 = mybir.dt.bfloat16
    AF = mybir.ActivationFunctionType

    f_dr = nc.dram_tensor("f", [NLOC, M * D], FP32, kind="ExternalInput")
    wt_dr = nc.dram_tensor("wt", [D, M * E], FP32, kind="ExternalInput")
    e4_dr = nc.dram_tensor("e4", [D, NG * NPAIR], BF16, kind="ExternalInput")
    madd_dr = nc.dram_tensor("madd", [NPAIR, 4 * NP2], FP32, kind="ExternalInput")
    minc_dr = nc.dram_tensor("minc", [NP2, M], FP32, kind="ExternalInput")
    eye_dr = nc.dram_tensor("eye", [128, 128], FP32, kind="ExternalInput")
    y_dr = nc.dram_tensor("y", [NLOC, E], FP32, kind="ExternalOutput")

    with tile.TileContext(nc) as tc:
        with (
            tc.tile_pool(name="const", bufs=1) as cpool,
            tc.tile_pool(name="fin", bufs=2) as fpool,
            tc.tile_pool(name="work", bufs=2) as wpool,
            tc.tile_pool(name="persist", bufs=1) as ppool,
            tc.tile_pool(name="ps_a", bufs=1, space=bass.MemorySpace.PSUM) as ps_a,
            tc.tile_pool(name="ps_b", bufs=1, space=bass.MemorySpace.PSUM) as ps_b,
            tc.tile_pool(name="ps_c", bufs=1, space=bass.MemorySpace.PSUM) as ps_c,
            tc.tile_pool(name="ps_d", bufs=2, space=bass.MemorySpace.PSUM) as ps_d,
            tc.tile_pool(name="ps_e", bufs=1, space=bass.MemorySpace.PSUM) as ps_e,
        ):
            # ---- constants in SBUF ----
            wt_sb = cpool.tile([D, M * E], FP32, tag="wt")
            e4_sb = cpool.tile([D, NG, NPAIR], BF16, tag="e4")
            madd_sb = cpool.tile([NPAIR, 4, NP2], FP32, tag="madd")
            minc_sb = cpool.tile([NP2, M], FP32, tag="minc")
            eye_sb = cpool.tile([128, 128], FP32, tag="eye")
            nc.sync.dma_start(wt_sb[:], wt_dr[:])
            nc.sync.dma_start(e4_sb[:].rearrange("p a b -> p (a b)"), e4_dr[:])
            nc.sync.dma_start(madd_sb[:].rearrange("p a b -> p (a b)"), madd_dr[:])
            nc.sync.dma_start(minc_sb[:], minc_dr[:])
            nc.sync.dma_start(eye_sb[:], eye_dr[:])

            # ---- persistent per-core state ----
            g_all = ppool.tile([128, NT, M, E], FP32, tag="g_all")
            hs_all = ppool.tile([128, NT, E], FP32, tag="hs_all")
            r_all = ppool.tile([128, NT, 128], FP32, tag="r_all")  # rows 0:36 used

            # ---- PSUM ----
            fT_ps = ps_a.tile([128, 4, 128], FP32, tag="fT")
            g_ps = ps_b.tile([128, 4, 128], FP32, tag="g")
            gT_ps = ps_c.tile([128, 4, 128], FP32, tag="gT")
            small_ps = ps_e.tile([128, 4, 128], FP32, tag="small")
            s_ps = small_ps[:, 0, :]
            c_ps = small_ps[:, 1, :]
            cT_ps = small_ps[:, 2, :]

            # =============== Phase A: per ntile ===============
            for nt in range(NT):
                f_sb = fpool.tile([128, M * D], FP32, tag="f")
                nc.sync.dma_start(f_sb[:], f_dr[nt * 128:(nt + 1) * 128, :])

                # hs = 0.5 * sum_m f_m   (Pool)
                hsr = wpool.tile([128, E], FP32, tag="hsr")
                nc.gpsimd.tensor_add(hsr[:], f_sb[:, 0:D], f_sb[:, D:2 * D])
                for m in range(2, M):
                    nc.gpsimd.tensor_add(hsr[:], hsr[:], f_sb[:, m * D:(m + 1) * D])
                nc.gpsimd.tensor_scalar_mul(hs_all[:, nt, :], hsr[:], 0.5)

                fT_sb = wpool.tile([128, M, D], FP32, tag="fT")
                gT_sb = wpool.tile([128, M, E], BF16, tag="gT")
                for m in range(M):
                    sl = m % 4
                    # fT_m = f_m.T  (PE transpose) -> SBUF via ACT copy
                    nc.tensor.transpose(fT_ps[:, sl, :], f_sb[:, m * D:(m + 1) * D], eye_sb[:])
                    nc.scalar.activation(fT_sb[:, m, :], fT_ps[:, sl, :], AF.Copy)
                    # g_m[n,e]  = fT_m.T @ WT_m   -> relu -> g_all (Pool, fp32)
                    nc.tensor.matmul(g_ps[:, sl, :], fT_sb[:, m, :], wt_sb[:, m * E:(m + 1) * E])
                    nc.scalar.activation(g_all[:, nt, m, :], g_ps[:, sl, :], AF.Relu)
                    # gT_m[e,n] = WT_m.T @ fT_m   -> relu -> gT_sb (DVE, bf16)
                    nc.tensor.matmul(gT_ps[:, sl, :], wt_sb[:, m * E:(m + 1) * E], fT_sb[:, m, :])
                    nc.vector.tensor_relu(gT_sb[:, m, :], gT_ps[:, sl, :])

                # triangular products H[d, t, n] = gT_j * gT_k  (DVE, bf16)
                h_sb = wpool.tile([128, NPAIR + 3, 128], BF16, tag="h")
                nc.vector.memset(h_sb[:, NPAIR:NPAIR + 3, :], 0.0)
                for j in range(M):
                    nj = M - j
                    in0 = gT_sb[:, j, :].unsqueeze(1).broadcast_to([128, nj, 128])
                    nc.vector.tensor_mul(h_sb[:, _OFF[j]:_OFF[j] + nj, :], in0, gT_sb[:, j:M, :])

                # dots[t, n] blocks via 12 accumulating selector matmuls (PE)
                dots_ps = ps_d.tile([128, 512], FP32, tag="dots")
                for g in range(NG):
                    nc.tensor.matmul(
                        dots_ps[0:NPAIR, :],
                        e4_sb[:, g, :],
                        h_sb[:, 4 * g:4 * g + 4, :],
                        start=(g == 0),
                        stop=(g == NG - 1),
                    )
                dots_sb = wpool.tile([128, 512], FP32, tag="dots_sb")
                nc.vector.tensor_copy(dots_sb[0:NPAIR, :], dots_ps[0:NPAIR, :])

                # s[p, n] = q_j + q_k - 2 D_jk  via 4 accumulating matmuls (PE)
                for c in range(4):
                    nc.tensor.matmul(
                        s_ps[0:NP2, :],
                        madd_sb[0:NPAIR, c, :],
                        dots_sb[0:NPAIR, c * 128:(c + 1) * 128],
                        start=(c == 0),
                        stop=(c == 3),
                    )
                s0_sb = wpool.tile([128, 128], FP32, tag="s0")
                nc.scalar.activation(s0_sb[0:NP2, :], s_ps[0:NP2, :], AF.Relu)
                nc.scalar.activation(r_all[0:NP2, nt, :], s0_sb[0:NP2, :], AF.Sqrt)

            # =============== Phase C: tanh + combine ===============
            for nt in range(NT):
                e_sb = wpool.tile([128, 128], FP32, tag="e")
                nc.scalar.activation(e_sb[0:NP2, :], r_all[0:NP2, nt, :], AF.Tanh)
                # c[k, n] = Minc_half.T @ e   (PE)
                nc.tensor.matmul(c_ps[0:M, :], minc_sb[0:NP2, :], e_sb[0:NP2, :])
                c_sb = wpool.tile([128, 128], FP32, tag="c")
                nc.scalar.activation(c_sb[0:M, :], c_ps[0:M, :], AF.Copy)
                # cT[n, k]  (PE transpose)
                nc.tensor.transpose(cT_ps[0:128, 0:M], c_sb[0:M, :], eye_sb[0:M, 0:M])
                cT_sb = wpool.tile([128, M], FP32, tag="cT")
                nc.scalar.activation(cT_sb[:], cT_ps[0:128, 0:M], AF.Copy)

                # y = hs + sum_k c_k * g_k  (DVE STT chain)
                y_sb = wpool.tile([128, E], FP32, tag="y")
                acc0 = wpool.tile([128, E], FP32, tag="acc0")
                acc1 = wpool.tile([128, E], FP32, tag="acc1")
                accs = [acc0, acc1]
                prev = hs_all[:, nt, :]
                for k in range(M):
                    out = y_sb[:] if k == M - 1 else accs[k % 2][:]
                    nc.vector.scalar_tensor_tensor(
                        out, g_all[:, nt, k, :], cT_sb[:, k:k + 1], prev,
                        AluOpType.mult, AluOpType.add,
                    )
                    prev = out
                nc.sync.dma_start(y_dr[nt * 128:(nt + 1) * 128, :], y_sb[:])


def _build_nc():
    from concourse import bacc

    nc = bacc.Bacc(target_bir_lowering=False, debug=False)
    _emit(nc)
    nc.compile()
    return nc


def _prepare(f: np.ndarray, W: np.ndarray):
    import ml_dtypes

    f = np.ascontiguousarray(f, dtype=np.float32).reshape(N, M * D)
    wt = np.ascontiguousarray(
        np.transpose(np.asarray(W, np.float32), (2, 0, 1)).reshape(D, M * E)
    )
    e4, maddT, minc, eye = _host_consts()
    e4 = np.ascontiguousarray(e4.reshape(D, NG * NPAIR).astype(ml_dtypes.bfloat16))
    maddT = np.ascontiguousarray(maddT.reshape(NPAIR, 4 * NP2))
    base = {"wt": wt, "e4": e4, "madd": maddT, "minc": minc, "eye": eye}

    nc = _build_nc()
    in_maps = [
        dict(base, f=np.ascontiguousarray(f[c * NLOC:(c + 1) * NLOC]))
        for c in range(NCORES)
    ]
    return nc, in_maps


def _run(f: np.ndarray, W: np.ndarray, trace: bool = False):
    from concourse.bass_utils import run_bass_kernel_spmd

    nc, in_maps = _prepare(f, W)
    res = run_bass_kernel_spmd(nc, in_maps, list(range(NCORES)), trace=trace)
    out = np.concatenate([np.asarray(r["y"]) for r in res.results], axis=0)
    return np.ascontiguousarray(out.astype(np.float32)), res


def kernel(f: np.ndarray, W: np.ndarray) -> np.ndarray:
    out, _ = _run(f, W, trace=False)
    return out


if __name__ == "__main__":
    rng = np.random.default_rng(0)
    f = rng.standard_normal((N, M, D), dtype=np.float32)
    W = rng.standard_normal((M, E, D), dtype=np.float32)
    y = kernel(f=f, W=W)
    print("kernel out", y.shape, y.dtype, float(np.abs(y).mean()))


# revision 12
# speedup vs baseline: 837.2683x; 837.2683x over previous
"""Trainium2 Bass kernel for nn_Inter_RM_6940667150684 (gnn_message_passing).

Math (per example n):
  g[n,m,:] = relu(f[n,m,:] @ W[m].T)            (W[m,e,d], contract d)
  s[n,j,k] = ||g_j - g_k||^2
  edges    = tanh(sqrt(relu(s)))  (diag 0, symmetric)
  y[n]     = 0.5*sum_m f[n,m,:] + sum_k c_k[n]*g[n,k,:]
  c_k[n]   = 0.5*sum_{j!=k} tanh(sqrt(relu(s_jk)))

Sharding: pure data parallel over batch N=8192 -> 8 cores x 1024 rows.
v2: bf16 f/W/g, 4-wide batched PSUM->SBUF copies.
"""

import sys

sys.path.insert(0, "/opt/trn_rl_repo")

import numpy as np

N, M, D, E = 8192, 9, 128, 128
NCORES = 8
NLOC = N // NCORES          # 1024 rows per core
NT = NLOC // 128            # 8 ntiles of 128 examples
NPAIR = 45                  # triangular tiles (j<=k), squares included
NP2 = 36                    # strict pairs j<k
NG = 12                     # dot-mm groups of 4 tiles

_TRI = [(j, k) for j in range(M) for k in range(j, M)]
_OFF = [0]
for j in range(M):
    _OFF.append(_OFF[-1] + (M - j))
_SQ = [_OFF[j] for j in range(M)]                       # index of (j,j)
_PAIRS = [(t, jk) for t, jk in enumerate(_TRI) if jk[0] < jk[1]]


def _host_consts():
    e4 = np.zeros((D, NG, NPAIR), np.float32)
    for t in range(NPAIR):
        e4[:, t // 4, t] = 1.0
    maddT = np.zeros((NPAIR, 4, NP2), np.float32)
    minc = np.zeros((NP2, M), np.float32)
    for p, (t, (j, k)) in enumerate(_PAIRS):
        maddT[_SQ[j], _SQ[j] % 4, p] += 1.0
        maddT[_SQ[k], _SQ[k] % 4, p] += 1.0
        maddT[t, t % 4, p] += -2.0
        minc[p, j] = 0.5
        minc[p, k] = 0.5
    eye = np.eye(128, dtype=np.float32)
    return e4, maddT, minc, eye


def _emit(nc, reps=1):
    from concourse import bass, tile
    from concourse.alu_op_type import AluOpType

    mybir = bass.mybir
    FP32 = mybir.dt.float32
    BF16 = mybir.dt.bfloat16
    AF = mybir.ActivationFunctionType

    f_dr = nc.dram_tensor("f", [NLOC, M * D], BF16, kind="ExternalInput")
    wt_dr = nc.dram_tensor("wt", [D, M * E], BF16, kind="ExternalInput")
    e4_dr = nc.dram_tensor("e4", [D, NG * NPAIR], BF16, kind="ExternalInput")
    madd_dr = nc.dram_tensor("madd", [NPAIR, 4 * NP2], FP32, kind="ExternalInput")
    minc_dr = nc.dram_tensor("minc", [NP2, M], FP32, kind="ExternalInput")
    eye_dr = nc.dram_tensor("eye", [128, 128], BF16, kind="ExternalInput")
    y_dr = nc.dram_tensor("y", [NLOC, E], FP32, kind="ExternalOutput")

    GRPS = [(0, 4), (4, 8), (8, 9)]  # m-index groups for 4-wide batching

    with tile.TileContext(nc) as tc:
        with (
            tc.tile_pool(name="const", bufs=1) as cpool,
            tc.tile_pool(name="fin", bufs=2) as fpool,
            tc.tile_pool(name="work", bufs=2) as wpool,
            tc.tile_pool(name="persist", bufs=1) as ppool,
            tc.tile_pool(name="ps_a", bufs=2, space=bass.MemorySpace.PSUM) as ps_a,
            tc.tile_pool(name="ps_b", bufs=1, space=bass.MemorySpace.PSUM) as ps_b,
            tc.tile_pool(name="ps_c", bufs=1, space=bass.MemorySpace.PSUM) as ps_c,
            tc.tile_pool(name="ps_d", bufs=2, space=bass.MemorySpace.PSUM) as ps_d,
            tc.tile_pool(name="ps_e", bufs=1, space=bass.MemorySpace.PSUM) as ps_e,
        ):
            # ---- constants in SBUF ----
            wt_sb = cpool.tile([D, M * E], BF16, tag="wt")
            e4_sb = cpool.tile([D, NG, NPAIR], BF16, tag="e4")
            madd_sb = cpool.tile([NPAIR, 4, NP2], FP32, tag="madd")
            minc_sb = cpool.tile([NP2, M], FP32, tag="minc")
            eye_sb = cpool.tile([128, 128], BF16, tag="eye")
            nc.sync.dma_start(wt_sb[:], wt_dr[:])
            nc.sync.dma_start(e4_sb[:].rearrange("p a b -> p (a b)"), e4_dr[:])
            nc.sync.dma_start(madd_sb[:].rearrange("p a b -> p (a b)"), madd_dr[:])
            nc.sync.dma_start(minc_sb[:], minc_dr[:])
            nc.sync.dma_start(eye_sb[:], eye_dr[:])

            # ---- persistent per-core state ----
            g_all = ppool.tile([128, NT, M, E], BF16, tag="g_all")
            hs_all = ppool.tile([128, NT, E], FP32, tag="hs_all")
            r_all = ppool.tile([128, NT, 128], FP32, tag="r_all")  # rows 0:36 used

            # ---- small PSUM (phase C + s) ----
            small_ps = ps_e.tile([128, 3, 128], FP32, tag="small")
            s_ps = small_ps[:, 0, :]
            c_ps = small_ps[:, 1, :]
            cT_ps = ps_e.tile([128, M], BF16, tag="cTp")

            def _body():
                # =============== Phase A: per ntile ===============
                for nt in range(NT):
                    f_sb = fpool.tile([128, M * D], BF16, tag="f")
                    nc.sync.dma_start(f_sb[:], f_dr[nt * 128:(nt + 1) * 128, :])

                    # hs = 0.5 * sum_m f_m   (Pool)
                    hsr = wpool.tile([128, E], FP32, tag="hsr")
                    nc.gpsimd.tensor_add(hsr[:], f_sb[:, 0:D], f_sb[:, D:2 * D])
                    for m in range(2, M):
                        nc.gpsimd.tensor_add(hsr[:], hsr[:], f_sb[:, m * D:(m + 1) * D])
                    nc.gpsimd.tensor_scalar_mul(hs_all[:, nt, :], hsr[:], 0.5)

                    fT_sb = wpool.tile([128, M, D], BF16, tag="fT")
                    gT_sb = wpool.tile([128, M, E], BF16, tag="gT")
                    for a, b in GRPS:
                        nb = b - a
                        fT_ps = ps_a.tile([128, 4, 128], BF16, tag="fTp")
                        for i in range(nb):
                            nc.tensor.transpose(
                                fT_ps[:, i, :],
                                f_sb[:, (a + i) * D:(a + i + 1) * D],
                                eye_sb[:],
                            )
                        nc.scalar.activation(
                            fT_sb[:, a:b, :], fT_ps[:, 0:nb, :], AF.Copy
                        )
                        g_ps = ps_b.tile([128, 4, 128], FP32, tag="gp")
                        gT_ps = ps_c.tile([128, 4, 128], FP32, tag="gTp")
                        for i in range(nb):
                            m = a + i
                            nc.tensor.matmul(
                                g_ps[:, i, :], fT_sb[:, m, :],
                                wt_sb[:, m * E:(m + 1) * E],
                            )
                            nc.tensor.matmul(
                                gT_ps[:, i, :], wt_sb[:, m * E:(m + 1) * E],
                                fT_sb[:, m, :],
                            )
                        nc.scalar.activation(
                            g_all[:, nt, a:b, :], g_ps[:, 0:nb, :], AF.Relu
                        )
                        nc.vector.tensor_relu(gT_sb[:, a:b, :], gT_ps[:, 0:nb, :])

                    # triangular products H[d, t, n] = gT_j * gT_k  (DVE, bf16)
                    h_sb = wpool.tile([128, NPAIR + 3, 128], BF16, tag="h")
                    nc.vector.memset(h_sb[:, NPAIR:NPAIR + 3, :], 0.0)
                    for j in range(M):
                        nj = M - j
                        in0 = gT_sb[:, j, :].unsqueeze(1).broadcast_to([128, nj, 128])
                        nc.vector.tensor_mul(
                            h_sb[:, _OFF[j]:_OFF[j] + nj, :], in0, gT_sb[:, j:M, :]
                        )

                    # dots[t, n] blocks via 12 accumulating selector matmuls (PE)
                    dots_ps = ps_d.tile([128, 512], FP32, tag="dots")
                    for g in range(NG):
                        nc.tensor.matmul(
                            dots_ps[0:NPAIR, :],
                            e4_sb[:, g, :],
                            h_sb[:, 4 * g:4 * g + 4, :],
                            start=(g == 0),
                            stop=(g == NG - 1),
                        )
                    dots_sb = wpool.tile([128, 512], FP32, tag="dots_sb")
                    nc.vector.tensor_copy(dots_sb[0:NPAIR, :], dots_ps[0:NPAIR, :])

                    # s[p, n] = q_j + q_k - 2 D_jk  via 4 accumulating matmuls (PE)
                    for c in range(4):
                        nc.tensor.matmul(
                            s_ps[0:NP2, :],
                            madd_sb[0:NPAIR, c, :],
                            dots_sb[0:NPAIR, c * 128:(c + 1) * 128],
                            start=(c == 0),
                            stop=(c == 3),
                        )
                    s0_sb = wpool.tile([128, 128], FP32, tag="s0")
                    nc.scalar.activation(s0_sb[0:NP2, :], s_ps[0:NP2, :], AF.Relu)
                    nc.scalar.activation(r_all[0:NP2, nt, :], s0_sb[0:NP2, :], AF.Sqrt)

                # =============== Phase C: tanh + combine ===============
                for nt in range(NT):
                    e_sb = wpool.tile([128, 128], FP32, tag="e")
                    nc.scalar.activation(e_sb[0:NP2, :], r_all[0:NP2, nt, :], AF.Tanh)
                    # c[k, n] = Minc_half.T @ e   (PE)
                    nc.tensor.matmul(c_ps[0:M, :], minc_sb[0:NP2, :], e_sb[0:NP2, :])
                    c_sb = wpool.tile([128, 128], BF16, tag="c")
                    nc.scalar.activation(c_sb[0:M, :], c_ps[0:M, :], AF.Copy)
                    # cT[n, k]  (PE transpose, bf16)
                    nc.tensor.transpose(cT_ps[0:128, 0:M], c_sb[0:M, :], eye_sb[0:M, 0:M])
                    cT_sb = wpool.tile([128, M], BF16, tag="cT")
                    nc.scalar.activation(cT_sb[:], cT_ps[0:128, 0:M], AF.Copy)

                    # y = hs + sum_k c_k * g_k  (DVE STT chain)
                    y_sb = wpool.tile([128, E], FP32, tag="y")
                    acc0 = wpool.tile([128, E], FP32, tag="acc0")
                    acc1 = wpool.tile([128, E], FP32, tag="acc1")
                    accs = [acc0, acc1]
                    prev = hs_all[:, nt, :]
                    for k in range(M):
                        out = y_sb[:] if k == M - 1 else accs[k % 2][:]
                        nc.vector.scalar_tensor_tensor(
                            out, g_all[:, nt, k, :], cT_sb[:, k:k + 1], prev,
                            AluOpType.mult, AluOpType.add,
                        )
                        prev = out
                    nc.sync.dma_start(y_dr[nt * 128:(nt + 1) * 128, :], y_sb[:])

            for _ in range(reps):
                _body()


def _build_nc(reps=1):
    from concourse import bacc

    nc = bacc.Bacc(target_bir_lowering=False, debug=False)
    _emit(nc, reps=reps)
    nc.compile()
    return nc


def _prepare(f: np.ndarray, W: np.ndarray, reps=1):
    import ml_dtypes

    BF = ml_dtypes.bfloat16
    f = np.ascontiguousarray(
        np.asarray(f, np.float32).reshape(N, M * D).astype(BF)
    )
    wt = np.ascontiguousarray(
        np.transpose(np.asarray(W, np.float32), (2, 0, 1)).reshape(D, M * E).astype(BF)
    )
    e4, maddT, minc, eye = _host_consts()
    e4 = np.ascontiguousarray(e4.reshape(D, NG * NPAIR).astype(BF))
    maddT = np.ascontiguousarray(maddT.reshape(NPAIR, 4 * NP2))
    base = {"wt": wt, "e4": e4, "madd": maddT, "minc": minc,
            "eye": eye.astype(BF)}

    nc = _build_nc(reps=reps)
    in_maps = [
        dict(base, f=np.ascontiguousarray(f[c * NLOC:(c + 1) * NLOC]))
        for c in range(NCORES)
    ]
    return nc, in_maps


def _run(f: np.ndarray, W: np.ndarray, trace: bool = False):
    from concourse.bass_utils import run_bass_kernel_spmd

    nc, in_maps = _prepare(f, W)
    res = run_bass_kernel_spmd(nc, in_maps, list(range(NCORES)), trace=trace)
    out = np.concatenate([np.asarray(r["y"]) for r in res.results], axis=0)
    return np.ascontiguousarray(out.astype(np.float32)), res


def kernel(f: np.ndarray, W: np.ndarray) -> np.ndarray:
    out, _ = _run(f, W, trace=False)
    return out


if __name__ == "__main__":
    rng = np.random.default_rng(0)
    f = rng.standard_normal((N, M, D), dtype=np.float32)
    W = rng.standard_normal((M, E, D), dtype=np.float32)
    y = kernel(f=f, W=W)
    print("kernel out", y.shape, y.dtype, float(np.abs(y).mean()))
